# revision 1
# baseline (speedup 1.0000x reference)
"""Trainium2 Bass kernel for ConvPixelToCapsules (conv -> 3-iter dynamic routing).

Strategy (hardcoded for x[8,32,8,32,32], conv_w[256,8,3,3], bias[32,8,1,1]):
  - Host precomputes im2col patches per batch element, with an extra 33rd
    "channel" slot holding sum_ci(x) (conv linearity gives iteration-1's
    uniform-route preactivation for free), plus the weight matrix in
    [72, (no,co)] layout and a partition-broadcast bias tile.
  - 8 NeuronCores, data-parallel over batch: core k owns batch element k.
  - Per core: 8 tiles of 128 output pixels. Per tile: 33 matmuls
    (stationary = patches[72,128], moving = w[72,256]) put votes directly in
    [pixel-partition; (ci,no,co)] layout in PSUM -> SBUF. All routing math is
    then free-dim vector/scalar ops (softmax over co, reduce over ci, squash
    over no, distances over no) — votes never leave SBUF. Final activations
    are PE-transposed so the HBM write is fully contiguous.
  - v2: votes/products in bf16 (DVE 2x mode), reductions as in-place halving
    trees of bf16 tensor_tensor adds, PSUM evacuation on the scalar engine.
    ITER3_FP32 runs the last routing iteration's reduction in fp32.
  - sqrt inside squash is computed as exp(0.5*ln(x)) so the scalar engine
    only ever needs the exp/ln activation-table set (no table thrashing).
"""

import numpy as np

BS, CI, NI, H, W = 8, 32, 8, 32, 32
CO, NO = 32, 8
NPIX = H * W            # 1024
TILES = 8               # tiles of 128 pixels per batch element
TP = 128                # pixels per tile (on partitions)
K = 72                  # ni * 3 * 3 contraction
SLOTS = CI + 1          # 32 ci + xsum slot
OUTCH = NO * CO         # 256, (no, co) order

CFG = {
    "iter3": "mixed",      # "bf16" | "mixed" | "fp32" last-iteration precision
    "pair": True,          # interleave emission of tile pairs
    "bf16_conv": True,     # patches+weights in bf16 (PE 1 cyc/row vs 4)
    "skip_routing": False, # conv+evac only (bisection)
    "skip_iters23": False, # stop after iteration 1 (bisection)
    "skip_iter3": False,   # stop after iteration 2 (bisection)
    "evac": "act",         # "act" | "dve" | "split"
    "gpsimd": True,        # offload fp32 side-chain ops to the idle GPSIMD
    "big_bufs": 1,
    "pconv_bufs": 5,
}

_BUILt = {}


def _host_prep(x, conv_w, bias):
    x = np.asarray(x, np.float32)
    conv_w = np.asarray(conv_w, np.float32)
    bias = np.asarray(bias, np.float32)
    x_pad = np.pad(x, ((0, 0), (0, 0), (0, 0), (1, 1), (1, 1)))
    x_aug = np.concatenate([x_pad, x_pad.sum(1, keepdims=True)], axis=1)
    wv = np.lib.stride_tricks.sliding_window_view(x_aug, (3, 3), axis=(3, 4))
    if CFG["bf16_conv"]:
        import ml_dtypes
        cdt_np = ml_dtypes.bfloat16
    else:
        cdt_np = np.float32
    patches = np.ascontiguousarray(
        wv.transpose(0, 2, 5, 6, 1, 3, 4).reshape(BS, K, SLOTS, NPIX)
    ).astype(cdt_np)
    w_m = np.ascontiguousarray(
        conv_w.reshape(CO, NO, NI, 3, 3).transpose(2, 3, 4, 1, 0).reshape(K, OUTCH)
    ).astype(cdt_np)
    bias_bc = np.broadcast_to(
        bias[:, :, 0, 0].T.reshape(1, OUTCH), (128, OUTCH)
    ).astype(np.float32)
    ident = np.eye(128, dtype=np.float32)
    return patches, w_m, bias_bc, ident


def _build_nc():
    key = ("nc",) + tuple(sorted(CFG.items()))
    if key in _BUILt:
        return _BUILt[key]
    import concourse.bacc as bacc
    import concourse.tile as tile
    import concourse.mybir as mybir

    f32 = mybir.dt.float32
    bf16 = mybir.dt.bfloat16
    AF = mybir.ActivationFunctionType
    OP = mybir.AluOpType
    AX = mybir.AxisListType

    nc = bacc.Bacc("TRN2", target_bir_lowering=False, debug=False, num_devices=8)

    cdt = bf16 if CFG["bf16_conv"] else f32
    patches_d = nc.dram_tensor("patches", [K, SLOTS, NPIX], cdt, kind="ExternalInput")
    w_d = nc.dram_tensor("w", [K, OUTCH], cdt, kind="ExternalInput")
    bias_d = nc.dram_tensor("bias", [128, OUTCH], f32, kind="ExternalInput")
    ident_d = nc.dram_tensor("ident", [128, 128], f32, kind="ExternalInput")
    out_d = nc.dram_tensor("out", [2, 128, NPIX], f32, kind="ExternalOutput")

    with tile.TileContext(nc) as tc:
        with (
            tc.tile_pool(name="const", bufs=1) as const,
            tc.tile_pool(name="pat", bufs=3) as patp,
            tc.tile_pool(name="votes", bufs=4) as votesp,
            tc.tile_pool(name="big", bufs=3) as bigp,
            tc.tile_pool(name="state", bufs=3) as statep,
            tc.tile_pool(name="obuf", bufs=1) as obufp,
            tc.tile_pool(name="pconv", bufs=CFG["pconv_bufs"], space="PSUM") as pconv,
            tc.tile_pool(name="ptr", bufs=2, space="PSUM") as ptr,
        ):
            w_sb = const.tile([K, OUTCH], cdt)
            nc.sync.dma_start(w_sb[:], w_d.ap())
            bias_sb = const.tile([128, OUTCH], f32)
            nc.sync.dma_start(bias_sb[:], bias_d.ap())
            ident_sb = const.tile([128, 128], f32)
            nc.sync.dma_start(ident_sb[:], ident_d.ap())
            eps_sb = const.tile([128, 1], f32)
            nc.gpsimd.memset(eps_sb[:], 1e-30)
            bias_nc = bias_sb[:].rearrange("p (n c) -> p n c", n=NO)

            ob = [
                obufp.tile([128, NPIX], f32, tag=f"ob{h}", name=f"ob{h}")
                for h in range(2)
            ]

            def conv_tile(t):
                # votes for 128 pixels; Uxs slot first so iteration 1 can
                # start before the full evacuation; head tiles split the
                # PSUM evac across DVE+ACT to fill the pipeline-fill idle.
                pt = patp.tile([K, SLOTS, TP], cdt, tag="pt", name=f"pt{t}")
                nc.sync.dma_start(
                    pt[:, CI, :], patches_d.ap()[:, CI, t * TP : (t + 1) * TP]
                )
                nc.sync.dma_start(
                    pt[:, :CI, :], patches_d.ap()[:, :CI, t * TP : (t + 1) * TP]
                )
                U = votesp.tile([128, CI, NO, CO], bf16, tag="U", name=f"U{t}")
                Uxs = votesp.tile([128, OUTCH], f32, tag="Uxs", name=f"Uxs{t}")
                conv_tile.out[t] = (U, Uxs)
                head = t < 2
                for i, s in enumerate([CI] + list(range(CI))):
                    pv = pconv.tile([128, OUTCH], f32, tag="pv", name=f"pv{t}_{s}")
                    nc.tensor.matmul(
                        pv[:], pt[:, s, :], w_sb[:], start=True, stop=True
                    )
                    dst = (U[:, s].rearrange("p n c -> p (n c)")
                           if s < CI else Uxs[:])
                    ev = CFG["evac"]
                    if (ev == "dve" or (ev == "split" and s % 2 == 0)
                            or (head and i % 2 == 1)):
                        nc.vector.tensor_copy(dst, pv[:])
                    else:
                        nc.scalar.copy(dst, pv[:])
                    if i % 2 == 1:
                        yield
            conv_tile.out = {}

            def emit_out(t, V):
                Vf = V[:].rearrange("p n c -> p (n c)")
                for h in range(2):
                    tp = ptr.tile([128, 128], f32, tag="tp", name=f"tp{t}_{h}")
                    nc.tensor.transpose(
                        tp[:], Vf[:, h * 128 : (h + 1) * 128], ident_sb[:]
                    )
                    nc.scalar.copy(ob[h][:, t * TP : (t + 1) * TP], tp[:])
                    nc.sync.dma_start(
                        out_d.ap()[h][:, t * TP : (t + 1) * TP],
                        ob[h][:, t * TP : (t + 1) * TP],
                    )

            def squash(t, S, it, out_dtype):
                # S: [128, NO, CO] f32 preactivation -> V [128, NO, CO]
                sq = statep.tile([128, NO, CO], f32, tag="sq", name=f"sq{t}_{it}")
                eng = nc.gpsimd if CFG["gpsimd"] else nc.vector
                eng.tensor_mul(sq[:], S[:], S[:])
                nsq = statep.tile([128, CO], f32, tag="nsq", name=f"nsq{t}_{it}")
                nc.vector.tensor_reduce(
                    nsq[:], sq[:].transpose([0, 2, 1]), axis=AX.X, op=OP.add
                )
                yield
                lg = statep.tile([128, CO], f32, tag="lg", name=f"lg{t}_{it}")
                nc.scalar.activation(lg[:], nsq[:], AF.Ln, bias=eps_sb[:])
                sqr = statep.tile([128, CO], f32, tag="sqr", name=f"sqr{t}_{it}")
                nc.scalar.activation(sqr[:], lg[:], AF.Exp, scale=0.5)
                den = statep.tile([128, CO], f32, tag="den", name=f"den{t}_{it}")
                eng.tensor_scalar_add(den[:], nsq[:], 1.0)
                rcd = statep.tile([128, CO], f32, tag="rcd", name=f"rcd{t}_{it}")
                nc.vector.reciprocal(rcd[:], den[:])
                yield
                scl = statep.tile([128, CO], f32, tag="scl", name=f"scl{t}_{it}")
                nc.vector.tensor_mul(scl[:], sqr[:], rcd[:])
                V = statep.tile([128, NO, CO], out_dtype, tag=f"V{it}",
                                name=f"V{t}_{it}")
                nc.vector.tensor_mul(
                    V[:], S[:], scl[:].unsqueeze(1).broadcast_to([128, NO, CO])
                )
                yield
                squash.out = V

            def d_tree(tmp2, out_f32):
                # reduce over no (axis 2) of [128, CI, NO, CO] bf16
                for hh in (4, 2):
                    nc.vector.tensor_add(
                        tmp2[:, :, :hh], tmp2[:, :, :hh], tmp2[:, :, hh : 2 * hh]
                    )
                nc.vector.tensor_add(out_f32, tmp2[:, :, 0], tmp2[:, :, 1])

            def routing_tile(t, U, Uxs):
                if CFG["skip_routing"]:
                    Vd = statep.tile([128, NO, CO], f32, tag="S", name=f"Vd{t}")
                    nc.vector.tensor_copy(
                        Vd[:].rearrange("p n c -> p (n c)"), Uxs[:]
                    )
                    emit_out(t, Vd)
                    return
                L = statep.tile([128, CI, CO], f32, tag="L", name=f"L{t}")
                # ---- iteration 1: route is uniform 1/CI ----
                S1 = statep.tile([128, NO, CO], f32, tag="S", name=f"S1_{t}")
                nc.vector.scalar_tensor_tensor(
                    S1[:].rearrange("p n c -> p (n c)"), Uxs[:], 1.0 / CI,
                    bias_sb[:], op0=OP.mult, op1=OP.add,
                )
                yield
                yield from squash(t, S1, 1,
                                  f32 if CFG["skip_iters23"] else bf16)
                V1 = squash.out
                if CFG["skip_iters23"]:
                    emit_out(t, V1)
                    return
                tmp = bigp.tile([128, CI, NO, CO], bf16, tag="tmp",
                                name=f"tmpa{t}")
                nc.vector.tensor_mul(
                    tmp[:], U[:],
                    V1[:].unsqueeze(1).broadcast_to([128, CI, NO, CO]),
                )
                yield
                d_tree(tmp, L[:])   # logits start at 0 -> L = D directly
                yield
                V = None
                for it in ((2,) if CFG["skip_iter3"] else (2, 3)):
                    i3 = "" if it == 2 else CFG["iter3"]
                    # ---- softmax over co ----
                    E = statep.tile([128, CI, CO],
                                    f32 if i3 in ("fp32", "mixed") else bf16,
                                    tag="E", name=f"E{t}_{it}")
                    nc.scalar.activation(E[:], L[:], AF.Exp)
                    sume = statep.tile([128, CI], f32, tag="sume",
                                       name=f"sume{t}_{it}")
                    nc.vector.tensor_reduce(sume[:], E[:], axis=AX.X, op=OP.add)
                    rec = statep.tile([128, CI], f32, tag="rec",
                                      name=f"rec{t}_{it}")
                    nc.vector.reciprocal(rec[:], sume[:])
                    yield
                    if i3 == "fp32":
                        recx = rec
                    elif i3 == "mixed":
                        recx = rec      # f32 inputs, bf16 product below
                    else:
                        recx = statep.tile([128, CI], bf16, tag="recb",
                                           name=f"recb{t}_{it}")
                        nc.vector.tensor_copy(recx[:], rec[:])
                    R = statep.tile([128, CI, CO],
                                    f32 if i3 == "fp32" else bf16,
                                    tag="R", name=f"R{t}_{it}")
                    nc.vector.tensor_mul(
                        R[:], E[:],
                        recx[:].unsqueeze(2).broadcast_to([128, CI, CO]),
                    )
                    yield
                    # ---- preactivation: sum_ci R * U ----
                    S = statep.tile([128, NO, CO], f32, tag="S",
                                    name=f"S{t}_{it}")
                    if i3 == "fp32":
                        tmp3 = bigp.tile([128, CI, NO, CO], f32, tag="tmp",
                                         name=f"tmp3{t}")
                        nc.vector.tensor_mul(
                            tmp3[:], U[:],
                            R[:].unsqueeze(2).broadcast_to([128, CI, NO, CO]),
                        )
                        yield
                        nc.vector.tensor_reduce(
                            S[:], tmp3[:].transpose([0, 2, 3, 1]),
                            axis=AX.X, op=OP.add,
                        )
                        yield
                    elif i3 == "mixed":
                        # bf16 products at 2x, exact fp32 accumulation
                        tmp = bigp.tile([128, CI, NO, CO], bf16, tag="tmp",
                                        name=f"tmpm{t}")
                        nc.vector.tensor_mul(
                            tmp[:], U[:],
                            R[:].unsqueeze(2).broadcast_to([128, CI, NO, CO]),
                        )
                        yield
                        nc.vector.tensor_reduce(
                            S[:], tmp[:].transpose([0, 2, 3, 1]),
                            axis=AX.X, op=OP.add,
                        )
                        yield
                    else:
                        tmp = bigp.tile([128, CI, NO, CO], bf16, tag="tmp",
                                        name=f"tmpb{t}_{it}")
                        nc.vector.tensor_mul(
                            tmp[:], U[:],
                            R[:].unsqueeze(2).broadcast_to([128, CI, NO, CO]),
                        )
                        yield
                        for hh in (16, 8, 4, 2):
                            nc.vector.tensor_add(
                                tmp[:, :hh], tmp[:, :hh], tmp[:, hh : 2 * hh]
                            )
                        nc.vector.tensor_add(S[:], tmp[:, 0], tmp[:, 1])
                        yield
                    (nc.gpsimd if CFG["gpsimd"] else nc.vector).tensor_add(
                        S[:], S[:], bias_nc)
                    yield from squash(t, S, it,
                                      f32 if (it == 3 or CFG["skip_iter3"])
                                      else bf16)
                    V = squash.out
                    if it == 2:
                        # ---- distances -> logits ----
                        tmp = bigp.tile([128, CI, NO, CO], bf16, tag="tmp",
                                        name=f"tmpd{t}")
                        nc.vector.tensor_mul(
                            tmp[:], U[:],
                            V[:].unsqueeze(1).broadcast_to([128, CI, NO, CO]),
                        )
                        yield
                        D = statep.tile([128, CI, CO], f32, tag="E",
                                        name=f"D{t}")
                        d_tree(tmp, D[:])
                        (nc.gpsimd if CFG["gpsimd"] else nc.vector).tensor_add(
                            L[:], L[:], D[:])
                        yield
                emit_out(t, V)

            def drain(gens):
                alive = [g for g in gens if g is not None]
                while alive:
                    for g in list(alive):
                        try:
                            next(g)
                        except StopIteration:
                            alive.remove(g)

            if CFG["pair"]:
                # pair 0 conv up front; each pair's routing drains together
                # with the NEXT pair's conv generators so the ACT evacuation
                # never head-of-line-blocks behind routing ACT ops
                drain([conv_tile(0), conv_tile(1)])
                for p in range(TILES // 2):
                    ts_ = (2 * p, 2 * p + 1)
                    gens = [routing_tile(t, *conv_tile.out[t]) for t in ts_]
                    if p + 1 < TILES // 2:
                        gens += [conv_tile(2 * p + 2), conv_tile(2 * p + 3)]
                    drain(gens)
            else:
                for t in range(TILES):
                    drain([conv_tile(t)])
                    drain([routing_tile(t, *conv_tile.out[t])])



    nc.compile()
    _BUILt[key] = nc
    return nc


def _assemble(out_halves_all):
    o = out_halves_all.reshape(-1, 2, 4, CO, NPIX)
    return np.ascontiguousarray(
        o.transpose(0, 3, 1, 2, 4).reshape(-1, CO, NO, H, W)
    )


def kernel(x, conv_w, bias):
    import sys
    if "/opt/trn_rl_repo" not in sys.path:
        sys.path.insert(0, "/opt/trn_rl_repo")
    from concourse import bass_utils

    patches, w_m, bias_bc, ident = _host_prep(x, conv_w, bias)
    nc = _build_nc()
    in_maps = [
        {"patches": patches[b], "w": w_m, "bias": bias_bc, "ident": ident}
        for b in range(BS)
    ]
    res = bass_utils.run_bass_kernel_spmd(nc, in_maps, core_ids=list(range(BS)))
    outs = np.stack([r["out"] for r in res.results])
    return _assemble(outs).astype(np.float32)



# revision 6
# speedup vs baseline: 1.0679x; 1.0679x over previous
"""Trainium2 Bass kernel for ConvPixelToCapsules (conv -> 3-iter dynamic routing).

Strategy (hardcoded for x[8,32,8,32,32], conv_w[256,8,3,3], bias[32,8,1,1]):
  - Host precomputes im2col patches per batch element, with an extra 33rd
    "channel" slot holding sum_ci(x) (conv linearity gives iteration-1's
    uniform-route preactivation for free), plus the weight matrix in
    [72, (no,co)] layout and a partition-broadcast bias tile.
  - 8 NeuronCores, data-parallel over batch: core k owns batch element k.
  - Per core: 8 tiles of 128 output pixels. Per tile: 33 matmuls
    (stationary = patches[72,128], moving = w[72,256]) put votes directly in
    [pixel-partition; (ci,no,co)] layout in PSUM -> SBUF. All routing math is
    then free-dim vector/scalar ops (softmax over co, reduce over ci, squash
    over no, distances over no) — votes never leave SBUF. Final activations
    are PE-transposed so the HBM write is fully contiguous.
  - v2: votes/products in bf16 (DVE 2x mode), reductions as in-place halving
    trees of bf16 tensor_tensor adds, PSUM evacuation on the scalar engine.
    ITER3_FP32 runs the last routing iteration's reduction in fp32.
  - sqrt inside squash is computed as exp(0.5*ln(x)) so the scalar engine
    only ever needs the exp/ln activation-table set (no table thrashing).
"""

import numpy as np

BS, CI, NI, H, W = 8, 32, 8, 32, 32
CO, NO = 32, 8
NPIX = H * W            # 1024
TILES = 8               # tiles of 128 pixels per batch element
TP = 128                # pixels per tile (on partitions)
K = 72                  # ni * 3 * 3 contraction
SLOTS = CI + 1          # 32 ci + xsum slot
OUTCH = NO * CO         # 256, (no, co) order

CFG = {
    "iter3": "bf16",       # "bf16" | "mixed" | "fp32" last-iteration precision
    "pair": True,          # interleave emission of tile pairs
    "bf16_conv": True,     # patches+weights in bf16 (PE 1 cyc/row vs 4)
    "skip_routing": False, # conv+evac only (bisection)
    "skip_iters23": False, # stop after iteration 1 (bisection)
    "skip_iter3": False,   # stop after iteration 2 (bisection)
    "evac": "act",         # "act" | "dve" | "split"
    "gpsimd": True,        # offload fp32 side-chain ops to the idle GPSIMD
    "big_bufs": 1,
    "pconv_bufs": 5,
    "act_preload": True,   # pre-load the combined Ln+Exp act table set once
    "ks": 20,              # ci slots of each big mul on DVE; rest on Pool
}

_BUILt = {}


def _host_prep(x, conv_w, bias):
    x = np.asarray(x, np.float32)
    conv_w = np.asarray(conv_w, np.float32)
    bias = np.asarray(bias, np.float32)
    x_pad = np.pad(x, ((0, 0), (0, 0), (0, 0), (1, 1), (1, 1)))
    x_aug = np.concatenate([x_pad, x_pad.sum(1, keepdims=True)], axis=1)
    wv = np.lib.stride_tricks.sliding_window_view(x_aug, (3, 3), axis=(3, 4))
    if CFG["bf16_conv"]:
        import ml_dtypes
        cdt_np = ml_dtypes.bfloat16
    else:
        cdt_np = np.float32
    patches = np.ascontiguousarray(
        wv.transpose(0, 2, 5, 6, 1, 3, 4).reshape(BS, K, SLOTS, NPIX)
    ).astype(cdt_np)
    w_m = np.ascontiguousarray(
        conv_w.reshape(CO, NO, NI, 3, 3).transpose(2, 3, 4, 1, 0).reshape(K, OUTCH)
    ).astype(cdt_np)
    bias_bc = np.broadcast_to(
        bias[:, :, 0, 0].T.reshape(1, OUTCH), (128, OUTCH)
    ).astype(np.float32)
    ident = np.eye(128, dtype=np.float32)
    return patches, w_m, bias_bc, ident


def _build_nc():
    key = ("nc",) + tuple(sorted(CFG.items()))
    if key in _BUILt:
        return _BUILt[key]
    import concourse.bacc as bacc
    import concourse.tile as tile
    import concourse.mybir as mybir

    f32 = mybir.dt.float32
    bf16 = mybir.dt.bfloat16
    AF = mybir.ActivationFunctionType
    OP = mybir.AluOpType
    AX = mybir.AxisListType

    nc = bacc.Bacc("TRN2", target_bir_lowering=False, debug=False, num_devices=8)

    cdt = bf16 if CFG["bf16_conv"] else f32
    patches_d = nc.dram_tensor("patches", [K, SLOTS, NPIX], cdt, kind="ExternalInput")
    w_d = nc.dram_tensor("w", [K, OUTCH], cdt, kind="ExternalInput")
    bias_d = nc.dram_tensor("bias", [128, OUTCH], f32, kind="ExternalInput")
    ident_d = nc.dram_tensor("ident", [128, 128], f32, kind="ExternalInput")
    out_d = nc.dram_tensor("out", [2, 128, NPIX], f32, kind="ExternalOutput")

    with tile.TileContext(nc) as tc:
        with (
            tc.tile_pool(name="const", bufs=1) as const,
            tc.tile_pool(name="pat", bufs=3) as patp,
            tc.tile_pool(name="votes", bufs=4) as votesp,
            tc.tile_pool(name="big", bufs=3) as bigp,
            tc.tile_pool(name="state", bufs=3) as statep,
            tc.tile_pool(name="obuf", bufs=1) as obufp,
            tc.tile_pool(name="pconv", bufs=CFG["pconv_bufs"], space="PSUM") as pconv,
            tc.tile_pool(name="ptr", bufs=2, space="PSUM") as ptr,
        ):
            if CFG["act_preload"]:
                # all activation funcs used (Copy/Identity/Square/Exp/Ln) live
                # in act set 6 (natural_log_exp_and_others); loading it up
                # front makes the fixpoint pass prove no further loads needed
                nc.scalar.add_instruction(mybir.InstLoadActFuncSet(
                    name=nc.get_next_instruction_name(), act_func_set_id=6,
                    ins=[], outs=[]))
            w_sb = const.tile([K, OUTCH], cdt)
            nc.sync.dma_start(w_sb[:], w_d.ap())
            bias_sb = const.tile([128, OUTCH], f32)
            nc.sync.dma_start(bias_sb[:], bias_d.ap())
            ident_sb = const.tile([128, 128], f32)
            nc.sync.dma_start(ident_sb[:], ident_d.ap())
            eps_sb = const.tile([128, 1], f32)
            nc.gpsimd.memset(eps_sb[:], 1e-30)
            bias_nc = bias_sb[:].rearrange("p (n c) -> p n c", n=NO)

            ob = [
                obufp.tile([128, NPIX], f32, tag=f"ob{h}", name=f"ob{h}")
                for h in range(2)
            ]

            def conv_tile(t):
                # votes for 128 pixels; Uxs slot first so iteration 1 can
                # start before the full evacuation; head tiles split the
                # PSUM evac across DVE+ACT to fill the pipeline-fill idle.
                pt = patp.tile([K, SLOTS, TP], cdt, tag="pt", name=f"pt{t}")
                nc.sync.dma_start(
                    pt[:, CI, :], patches_d.ap()[:, CI, t * TP : (t + 1) * TP]
                )
                nc.sync.dma_start(
                    pt[:, :CI, :], patches_d.ap()[:, :CI, t * TP : (t + 1) * TP]
                )
                U = votesp.tile([128, CI, NO, CO], bf16, tag="U", name=f"U{t}")
                Uxs = votesp.tile([128, OUTCH], f32, tag="Uxs", name=f"Uxs{t}")
                conv_tile.out[t] = (U, Uxs)
                head = t < 2
                for i, s in enumerate([CI] + list(range(CI))):
                    pv = pconv.tile([128, OUTCH], f32, tag="pv", name=f"pv{t}_{s}")
                    nc.tensor.matmul(
                        pv[:], pt[:, s, :], w_sb[:], start=True, stop=True
                    )
                    dst = (U[:, s].rearrange("p n c -> p (n c)")
                           if s < CI else Uxs[:])
                    ev = CFG["evac"]
                    if (ev == "dve" or (ev == "split" and s % 2 == 0)
                            or (head and i % 2 == 1)):
                        nc.vector.tensor_copy(dst, pv[:])
                    else:
                        nc.scalar.copy(dst, pv[:])
                    if i % 2 == 1:
                        yield
            conv_tile.out = {}

            def emit_out(t, V):
                Vf = V[:].rearrange("p n c -> p (n c)")
                for h in range(2):
                    tp = ptr.tile([128, 128], f32, tag="tp", name=f"tp{t}_{h}")
                    nc.tensor.transpose(
                        tp[:], Vf[:, h * 128 : (h + 1) * 128], ident_sb[:]
                    )
                    nc.scalar.copy(ob[h][:, t * TP : (t + 1) * TP], tp[:])
                    nc.sync.dma_start(
                        out_d.ap()[h][:, t * TP : (t + 1) * TP],
                        ob[h][:, t * TP : (t + 1) * TP],
                    )

            def squash(t, S, it, out_dtype):
                # S: [128, NO, CO] f32 preactivation -> V [128, NO, CO]
                sq = statep.tile([128, NO, CO], f32, tag="sq", name=f"sq{t}_{it}")
                nc.scalar.activation(sq[:], S[:], AF.Square)
                nsq = statep.tile([128, CO], f32, tag="nsq", name=f"nsq{t}_{it}")
                nc.vector.tensor_reduce(
                    nsq[:], sq[:].transpose([0, 2, 1]), axis=AX.X, op=OP.add
                )
                yield
                lg = statep.tile([128, CO], f32, tag="lg", name=f"lg{t}_{it}")
                nc.scalar.activation(lg[:], nsq[:], AF.Ln, bias=eps_sb[:])
                sqr = statep.tile([128, CO], f32, tag="sqr", name=f"sqr{t}_{it}")
                nc.scalar.activation(sqr[:], lg[:], AF.Exp, scale=0.5)
                den = statep.tile([128, CO], f32, tag="den", name=f"den{t}_{it}")
                nc.gpsimd.tensor_scalar_add(den[:], nsq[:], 1.0)
                rcd = statep.tile([128, CO], f32, tag="rcd", name=f"rcd{t}_{it}")
                nc.vector.reciprocal(rcd[:], den[:])
                yield
                scl = statep.tile([128, CO], f32, tag="scl", name=f"scl{t}_{it}")
                nc.vector.tensor_mul(scl[:], sqr[:], rcd[:])
                V = statep.tile([128, NO, CO], out_dtype, tag=f"V{it}",
                                name=f"V{t}_{it}")
                nc.gpsimd.tensor_mul(
                    V[:], S[:], scl[:].unsqueeze(1).broadcast_to([128, NO, CO])
                )
                yield
                squash.out = V

            def d_tree(tmp2, out_f32):
                # reduce over no (axis 2) of [128, CI, NO, CO] bf16
                for hh in (4, 2):
                    nc.vector.tensor_add(
                        tmp2[:, :, :hh], tmp2[:, :, :hh], tmp2[:, :, hh : 2 * hh]
                    )
                nc.vector.tensor_add(out_f32, tmp2[:, :, 0], tmp2[:, :, 1])

            KS = CFG["ks"]

            def bigmul(tmp, U, bc):
                # U[128,CI,NO,CO] * broadcast factor, ci-split DVE/Pool.
                # bc(lo, hi) returns the broadcast operand for ci slice lo:hi.
                if KS < CI:
                    nc.gpsimd.tensor_mul(tmp[:, KS:], U[:, KS:], bc(KS, CI))
                nc.vector.tensor_mul(tmp[:, :KS], U[:, :KS], bc(0, KS))

            def routing_tile(t, U, Uxs):
                if CFG["skip_routing"]:
                    Vd = statep.tile([128, NO, CO], f32, tag="S", name=f"Vd{t}")
                    nc.vector.tensor_copy(
                        Vd[:].rearrange("p n c -> p (n c)"), Uxs[:]
                    )
                    emit_out(t, Vd)
                    return
                L = statep.tile([128, CI, CO], f32, tag="L", name=f"L{t}")
                # ---- iteration 1: route is uniform 1/CI ----
                S1 = statep.tile([128, NO, CO], f32, tag="S", name=f"S1_{t}")
                nc.vector.scalar_tensor_tensor(
                    S1[:].rearrange("p n c -> p (n c)"), Uxs[:], 1.0 / CI,
                    bias_sb[:], op0=OP.mult, op1=OP.add,
                )
                yield
                yield from squash(t, S1, 1,
                                  f32 if CFG["skip_iters23"] else bf16)
                V1 = squash.out
                if CFG["skip_iters23"]:
                    emit_out(t, V1)
                    return
                tmp = bigp.tile([128, CI, NO, CO], bf16, tag="tmp",
                                name=f"tmpa{t}")
                bigmul(tmp, U, lambda lo, hi: V1[:].unsqueeze(1)
                       .broadcast_to([128, hi - lo, NO, CO]))
                yield
                d_tree(tmp, L[:])   # logits start at 0 -> L = D directly
                yield
                V = None
                for it in ((2,) if CFG["skip_iter3"] else (2, 3)):
                    # ---- softmax over co ----
                    E = statep.tile([128, CI, CO], bf16,
                                    tag="E", name=f"E{t}_{it}")
                    nc.scalar.activation(E[:], L[:], AF.Exp)
                    sume = statep.tile([128, CI], f32, tag="sume",
                                       name=f"sume{t}_{it}")
                    nc.vector.tensor_reduce(sume[:], E[:], axis=AX.X, op=OP.add)
                    rec = statep.tile([128, CI], f32, tag="rec",
                                      name=f"rec{t}_{it}")
                    nc.vector.reciprocal(rec[:], sume[:])
                    yield
                    R = statep.tile([128, CI, CO], bf16,
                                    tag="R", name=f"R{t}_{it}")
                    nc.gpsimd.tensor_mul(
                        R[:], E[:],
                        rec[:].unsqueeze(2).broadcast_to([128, CI, CO]),
                    )
                    yield
                    # ---- preactivation: sum_ci R * U ----
                    S = statep.tile([128, NO, CO], f32, tag="S",
                                    name=f"S{t}_{it}")
                    tmp = bigp.tile([128, CI, NO, CO], bf16, tag="tmp",
                                    name=f"tmpb{t}_{it}")
                    bigmul(tmp, U, lambda lo, hi: R[:, lo:hi].unsqueeze(2)
                           .broadcast_to([128, hi - lo, NO, CO]))
                    yield
                    for hh in (16, 8, 4, 2):
                        nc.vector.tensor_add(
                            tmp[:, :hh], tmp[:, :hh], tmp[:, hh : 2 * hh]
                        )
                    nc.vector.tensor_add(S[:], tmp[:, 0], tmp[:, 1])
                    yield
                    nc.gpsimd.tensor_add(S[:], S[:], bias_nc)
                    yield from squash(t, S, it,
                                      f32 if (it == 3 or CFG["skip_iter3"])
                                      else bf16)
                    V = squash.out
                    if it == 2:
                        # ---- distances -> logits ----
                        tmp = bigp.tile([128, CI, NO, CO], bf16, tag="tmp",
                                        name=f"tmpd{t}")
                        bigmul(tmp, U, lambda lo, hi: V[:].unsqueeze(1)
                               .broadcast_to([128, hi - lo, NO, CO]))
                        yield
                        D = statep.tile([128, CI, CO], f32, tag="E",
                                        name=f"D{t}")
                        d_tree(tmp, D[:])
                        nc.vector.tensor_add(L[:], L[:], D[:])
                        yield
                emit_out(t, V)

            def drain(gens):
                alive = [g for g in gens if g is not None]
                while alive:
                    for g in list(alive):
                        try:
                            next(g)
                        except StopIteration:
                            alive.remove(g)

            if CFG["pair"]:
                # pair 0 conv up front; each pair's routing drains together
                # with the NEXT pair's conv generators so the ACT evacuation
                # never head-of-line-blocks behind routing ACT ops
                drain([conv_tile(0), conv_tile(1)])
                for p in range(TILES // 2):
                    ts_ = (2 * p, 2 * p + 1)
                    gens = [routing_tile(t, *conv_tile.out[t]) for t in ts_]
                    if p + 1 < TILES // 2:
                        gens += [conv_tile(2 * p + 2), conv_tile(2 * p + 3)]
                    drain(gens)
            else:
                for t in range(TILES):
                    drain([conv_tile(t)])
                    drain([routing_tile(t, *conv_tile.out[t])])



    nc.compile()
    _BUILt[key] = nc
    return nc


def _assemble(out_halves_all):
    o = out_halves_all.reshape(-1, 2, 4, CO, NPIX)
    return np.ascontiguousarray(
        o.transpose(0, 3, 1, 2, 4).reshape(-1, CO, NO, H, W)
    )


def kernel(x, conv_w, bias):
    import sys
    if "/opt/trn_rl_repo" not in sys.path:
        sys.path.insert(0, "/opt/trn_rl_repo")
    from concourse import bass_utils

    patches, w_m, bias_bc, ident = _host_prep(x, conv_w, bias)
    nc = _build_nc()
    in_maps = [
        {"patches": patches[b], "w": w_m, "bias": bias_bc, "ident": ident}
        for b in range(BS)
    ]
    res = bass_utils.run_bass_kernel_spmd(nc, in_maps, core_ids=list(range(BS)))
    outs = np.stack([r["out"] for r in res.results])
    return _assemble(outs).astype(np.float32)



# revision 41
# speedup vs baseline: 1.3223x; 1.2382x over previous
"""Trainium2 Bass kernel for ConvPixelToCapsules (conv -> 3-iter dynamic routing).

Strategy (hardcoded for x[8,32,8,32,32], conv_w[256,8,3,3], bias[32,8,1,1]):
  - Host precomputes im2col patches per batch element, with an extra 33rd
    "channel" slot holding sum_ci(x) (conv linearity gives iteration-1's
    uniform-route preactivation for free), plus the weight matrix in
    [72, (no,co)] layout and a partition-broadcast bias tile.
  - 8 NeuronCores, data-parallel over batch: core k owns batch element k.
  - Per core: 8 tiles of 128 output pixels. Per tile: 33 matmuls
    (stationary = patches[72,128], moving = w[72,256]) put votes directly in
    [pixel-partition; (ci,no,co)] layout in PSUM -> SBUF. All routing math is
    then free-dim vector/scalar ops (softmax over co, reduce over ci, squash
    over no, distances over no) — votes never leave SBUF. Final activations
    are PE-transposed so the HBM write is fully contiguous.
  - v2: votes/products in bf16 (DVE 2x mode), reductions as in-place halving
    trees of bf16 tensor_tensor adds, PSUM evacuation on the scalar engine.
    ITER3_FP32 runs the last routing iteration's reduction in fp32.
  - sqrt inside squash is computed as exp(0.5*ln(x)) so the scalar engine
    only ever needs the exp/ln activation-table set (no table thrashing).
"""

import numpy as np

BS, CI, NI, H, W = 8, 32, 8, 32, 32
CO, NO = 32, 8
NPIX = H * W            # 1024
TILES = 8               # tiles of 128 pixels per batch element
TP = 128                # pixels per tile (on partitions)
K = 72                  # ni * 3 * 3 contraction
KP = K + 1              # + constant row carrying the bias into the xsum slot
SLOTS = CI + 1          # 32 ci + xsum slot
OUTCH = NO * CO         # 256, (no, co) order

CFG = {
    "iter3": "bf16",       # "bf16" | "mixed" | "fp32" last-iteration precision
    "pair": True,          # interleave emission of tile pairs
    "bf16_conv": True,     # patches+weights in bf16 (PE 1 cyc/row vs 4)
    "skip_routing": False, # conv+evac only (bisection)
    "skip_iters23": False, # stop after iteration 1 (bisection)
    "skip_iter3": False,   # stop after iteration 2 (bisection)
    "evac": "act",         # "act" | "dve" | "split"
    "gpsimd": True,        # offload fp32 side-chain ops to the idle GPSIMD
    "big_bufs": 3,
    "pconv_bufs": 5,
    "act_preload": True,   # pre-load the combined Ln+Exp act table set once
    "cspl": 25,            # co columns owned by DVE in the products
    "cspl_tree": 27,       # co columns owned by DVE in tree levels
    "r_pool": False,       # softmax renorm mul on Pool
    "group": 2,            # tiles routed concurrently (pipeline depth)
    "votes_bufs": 4,
    "state_bufs": 5,
    "pat_bufs": 2,
    "ptr_bufs": 2,
    "obuf": False,         # False: stage transposes via small per-tile tiles
}

_BUILt = {}


def _host_prep(x, conv_w, bias):
    x = np.asarray(x, np.float32)
    conv_w = np.asarray(conv_w, np.float32)
    bias = np.asarray(bias, np.float32)
    x_pad = np.pad(x, ((0, 0), (0, 0), (0, 0), (1, 1), (1, 1)))
    # xsum slot pre-scaled by 1/CI so the conv emits iteration-1's
    # preactivation directly; the extra constant row adds the bias
    x_aug = np.concatenate(
        [x_pad, x_pad.sum(1, keepdims=True) * (1.0 / CI)], axis=1)
    wv = np.lib.stride_tricks.sliding_window_view(x_aug, (3, 3), axis=(3, 4))
    if CFG["bf16_conv"]:
        import ml_dtypes
        cdt_np = ml_dtypes.bfloat16
    else:
        cdt_np = np.float32
    patches = np.zeros((BS, KP, SLOTS, NPIX), dtype=cdt_np)
    patches[:, :K] = np.ascontiguousarray(
        wv.transpose(0, 2, 5, 6, 1, 3, 4).reshape(BS, K, SLOTS, NPIX)
    ).astype(cdt_np)
    patches[:, K, CI, :] = cdt_np(1.0)   # const row active only in xsum slot
    bias_flat = bias[:, :, 0, 0].T.reshape(OUTCH)     # (no, co) order
    w_m = np.zeros((KP, OUTCH), dtype=cdt_np)
    w_m[:K] = np.ascontiguousarray(
        conv_w.reshape(CO, NO, NI, 3, 3).transpose(2, 3, 4, 1, 0).reshape(K, OUTCH)
    ).astype(cdt_np)
    w_m[K] = bias_flat.astype(cdt_np)
    bias_bc = np.broadcast_to(
        bias_flat.reshape(1, OUTCH), (128, OUTCH)
    ).astype(np.float32)
    ident = np.eye(128, dtype=np.float32)
    return patches, w_m, bias_bc, ident


def _build_nc():
    key = ("nc",) + tuple(sorted(CFG.items()))
    if key in _BUILt:
        return _BUILt[key]
    import concourse.bacc as bacc
    import concourse.tile as tile
    import concourse.mybir as mybir

    f32 = mybir.dt.float32
    bf16 = mybir.dt.bfloat16
    AF = mybir.ActivationFunctionType
    OP = mybir.AluOpType
    AX = mybir.AxisListType

    nc = bacc.Bacc("TRN2", target_bir_lowering=False, debug=False, num_devices=8)

    cdt = bf16 if CFG["bf16_conv"] else f32
    patches_d = nc.dram_tensor("patches", [KP, SLOTS, NPIX], cdt, kind="ExternalInput")
    w_d = nc.dram_tensor("w", [KP, OUTCH], cdt, kind="ExternalInput")
    bias_d = nc.dram_tensor("bias", [128, OUTCH], f32, kind="ExternalInput")
    ident_d = nc.dram_tensor("ident", [128, 128], f32, kind="ExternalInput")
    out_d = nc.dram_tensor("out", [2, 128, NPIX], f32, kind="ExternalOutput")

    with tile.TileContext(nc) as tc:
        with (
            tc.tile_pool(name="const", bufs=1) as const,
            tc.tile_pool(name="pat", bufs=CFG["pat_bufs"]) as patp,
            tc.tile_pool(name="votes", bufs=CFG["votes_bufs"]) as votesp,
            tc.tile_pool(name="big", bufs=CFG["big_bufs"]) as bigp,
            tc.tile_pool(name="state", bufs=CFG["state_bufs"]) as statep,
            tc.tile_pool(name="obuf", bufs=1 if CFG["obuf"] else 4) as obufp,
            tc.tile_pool(name="pconv", bufs=CFG["pconv_bufs"], space="PSUM") as pconv,
            tc.tile_pool(name="pvxp", bufs=1, space="PSUM") as pvxp,
            tc.tile_pool(name="ptr", bufs=CFG.get("ptr_bufs", 2), space="PSUM") as ptr,
        ):
            if CFG["act_preload"]:
                # all activation funcs used (Copy/Identity/Square/Exp/Ln) live
                # in act set 6 (natural_log_exp_and_others); loading it up
                # front makes the fixpoint pass prove no further loads needed
                nc.scalar.add_instruction(mybir.InstLoadActFuncSet(
                    name=nc.get_next_instruction_name(), act_func_set_id=6,
                    ins=[], outs=[]))
            w_sb = const.tile([KP, OUTCH], cdt)
            nc.sync.dma_start(w_sb[:], w_d.ap())
            bias_sb = const.tile([128, OUTCH], f32)
            nc.sync.dma_start(bias_sb[:], bias_d.ap())
            ident_sb = const.tile([128, 128], f32)
            nc.sync.dma_start(ident_sb[:], ident_d.ap())
            eps_sb = const.tile([128, 1], f32)
            nc.gpsimd.memset(eps_sb[:], 1e-30)
            bias_nc = bias_sb[:].rearrange("p (n c) -> p n c", n=NO)

            ob = [
                obufp.tile([128, NPIX], f32, tag=f"ob{h}", name=f"ob{h}")
                for h in range(2)
            ] if CFG["obuf"] else None

            def conv_tile(t):
                # votes for 128 pixels; Uxs slot first so iteration 1 can
                # start before the full evacuation. ci slots evacuate in
                # chunks of 4 (one wide ACT copy per 4 matmuls); head tiles
                # alternate chunks onto DVE to fill the pipeline-fill idle.
                pt = patp.tile([KP, SLOTS, TP], cdt, tag="pt", name=f"pt{t}")
                nc.sync.dma_start(
                    pt[:, CI, :], patches_d.ap()[:, CI, t * TP : (t + 1) * TP]
                )
                nc.sync.dma_start(
                    pt[:, :CI, :], patches_d.ap()[:, :CI, t * TP : (t + 1) * TP]
                )
                U = votesp.tile([128, CI, NO, CO], bf16, tag="U", name=f"U{t}")
                Uxs = votesp.tile([128, OUTCH], f32, tag="Uxs", name=f"Uxs{t}")
                conv_tile.out[t] = (U, Uxs)
                head = t < 2
                for i, s in enumerate([CI] + list(range(CI))):
                    pv = pconv.tile([128, OUTCH], f32, tag="pv",
                                    name=f"pv{t}_{s}")
                    nc.tensor.matmul(
                        pv[:], pt[:, s, :], w_sb[:], start=True, stop=True
                    )
                    dst = (U[:, s].rearrange("p n c -> p (n c)")
                           if s < CI else Uxs[:])
                    if head and i % 2 == 1:
                        nc.vector.tensor_copy(dst, pv[:])
                    else:
                        nc.scalar.copy(dst, pv[:])
                    if i % 2 == 1:
                        yield
            conv_tile.out = {}

            def emit_out(t, V):
                Vf = V[:].rearrange("p n c -> p (n c)")
                for h in range(2):
                    tp = ptr.tile([128, 128], f32, tag="tp", name=f"tp{t}_{h}")
                    nc.tensor.transpose(
                        tp[:], Vf[:, h * 128 : (h + 1) * 128], ident_sb[:]
                    )
                    if CFG["obuf"]:
                        nc.scalar.copy(ob[h][:, t * TP : (t + 1) * TP], tp[:])
                        nc.sync.dma_start(
                            out_d.ap()[h][:, t * TP : (t + 1) * TP],
                            ob[h][:, t * TP : (t + 1) * TP],
                        )
                    else:
                        st = obufp.tile([128, TP], f32, tag="ostage",
                                        name=f"ost{t}_{h}")
                        nc.scalar.copy(st[:], tp[:])
                        nc.sync.dma_start(
                            out_d.ap()[h][:, t * TP : (t + 1) * TP], st[:]
                        )

            def squash(t, S, it, out_dtype):
                # S: [128, NO, CO] f32 preactivation AP -> V [128, NO, CO]
                # Square written transposed (ACT cost is stride-blind) so the
                # norm reduce runs packed bf16 at 2x
                sq = statep.tile([128, CO, NO], bf16, tag="sq",
                                 name=f"sq{t}_{it}")
                nc.scalar.activation(sq[:].transpose([0, 2, 1]), S, AF.Square)
                nsq = statep.tile([128, CO], bf16, tag="nsq", name=f"nsq{t}_{it}")
                with nc.allow_low_precision("nsq: norms O(1), bf16 ok"):
                    nc.vector.tensor_reduce(nsq[:], sq[:], axis=AX.X, op=OP.add)
                yield
                lg = statep.tile([128, CO], f32, tag="lg", name=f"lg{t}_{it}")
                nc.scalar.activation(lg[:], nsq[:], AF.Ln, bias=eps_sb[:])
                sqr = statep.tile([128, CO], f32, tag="sqr", name=f"sqr{t}_{it}")
                nc.scalar.activation(sqr[:], lg[:], AF.Exp, scale=0.5)
                den = statep.tile([128, CO], f32, tag="den", name=f"den{t}_{it}")
                nc.gpsimd.tensor_scalar_add(den[:], nsq[:], 1.0)
                rcd = statep.tile([128, CO], f32, tag="rcd", name=f"rcd{t}_{it}")
                nc.vector.reciprocal(rcd[:], den[:])
                yield
                scl = statep.tile([128, CO], f32, tag="scl", name=f"scl{t}_{it}")
                nc.vector.tensor_mul(scl[:], sqr[:], rcd[:])
                V = statep.tile([128, NO, CO], out_dtype, tag=f"V{it}",
                                name=f"V{t}_{it}")
                nc.gpsimd.tensor_mul(
                    V[:], S, scl[:].unsqueeze(1).broadcast_to([128, NO, CO])
                )
                yield
                squash.out = V

            CS = CFG["cspl"]
            CT = CFG["cspl_tree"]

            def d_tree(tmp2, out_ap):
                # reduce over no (axis 2) of [128, CI, NO, CO] bf16;
                # levels >= 2048 cols co-split across Pool/DVE
                for hh in (4, 2):
                    nc.gpsimd.tensor_add(
                        tmp2[:, :, :hh, CT:], tmp2[:, :, :hh, CT:],
                        tmp2[:, :, hh : 2 * hh, CT:]
                    )
                    nc.vector.tensor_add(
                        tmp2[:, :, :hh, :CT], tmp2[:, :, :hh, :CT],
                        tmp2[:, :, hh : 2 * hh, :CT]
                    )
                nc.vector.tensor_add(out_ap, tmp2[:, :, 0], tmp2[:, :, 1])

            def bigmul(tmp, U, bc):
                # U[128,CI,NO,CO] * broadcast factor, co-ownership split:
                # bc(lo, hi) returns the broadcast operand for co slice lo:hi.
                nc.gpsimd.tensor_mul(
                    tmp[:, :, :, CS:], U[:, :, :, CS:], bc(CS, CO))
                nc.vector.tensor_mul(
                    tmp[:, :, :, :CS], U[:, :, :, :CS], bc(0, CS))

            def routing_tile(t, U, Uxs):
                if CFG["skip_routing"]:
                    Vd = statep.tile([128, NO, CO], f32, tag="S", name=f"Vd{t}")
                    nc.vector.tensor_copy(
                        Vd[:].rearrange("p n c -> p (n c)"), Uxs[:]
                    )
                    emit_out(t, Vd)
                    return
                L = statep.tile([128, CI, CO], bf16, tag="L", name=f"L{t}")
                # ---- iteration 1: uniform route; S1 came out of the conv
                # (xsum slot pre-scaled 1/CI, bias via the constant row) ----
                S1 = Uxs[:].rearrange("p (n c) -> p n c", n=NO)
                yield from squash(t, S1, 1,
                                  f32 if CFG["skip_iters23"] else bf16)
                V1 = squash.out
                if CFG["skip_iters23"]:
                    emit_out(t, V1)
                    return
                tmp = bigp.tile([128, CI, NO, CO], bf16, tag="tmp",
                                name=f"tmpa{t}")
                bigmul(tmp, U, lambda lo, hi: V1[:, :, lo:hi].unsqueeze(1)
                       .broadcast_to([128, CI, NO, hi - lo]))
                yield
                d_tree(tmp, L[:])   # logits start at 0 -> L = D directly
                yield
                V = None
                for it in ((2,) if CFG["skip_iter3"] else (2, 3)):
                    # ---- softmax over co ----
                    E = statep.tile([128, CI, CO], bf16,
                                    tag="E", name=f"E{t}_{it}")
                    nc.scalar.activation(E[:], L[:], AF.Exp)
                    sume = statep.tile([128, CI], bf16, tag="sume",
                                       name=f"sume{t}_{it}")
                    with nc.allow_low_precision("sume: values in [29,41]"):
                        nc.vector.tensor_reduce(sume[:], E[:], axis=AX.X,
                                                op=OP.add)
                    rec = statep.tile([128, CI], f32, tag="rec",
                                      name=f"rec{t}_{it}")
                    nc.vector.reciprocal(rec[:], sume[:])
                    yield
                    R = statep.tile([128, CI, CO], bf16,
                                    tag="R", name=f"R{t}_{it}")
                    (nc.gpsimd if CFG["r_pool"] else nc.vector).tensor_mul(
                        R[:], E[:],
                        rec[:].unsqueeze(2).broadcast_to([128, CI, CO]),
                    )
                    yield
                    # ---- preactivation: sum_ci R * U ----
                    S = statep.tile([128, NO, CO], f32, tag="S",
                                    name=f"S{t}_{it}")
                    tmp = bigp.tile([128, CI, NO, CO], bf16, tag="tmp",
                                    name=f"tmpb{t}_{it}")
                    bigmul(tmp, U, lambda lo, hi: R[:, :, lo:hi].unsqueeze(2)
                           .broadcast_to([128, CI, NO, hi - lo]))
                    yield
                    for hh in (16, 8):
                        nc.gpsimd.tensor_add(
                            tmp[:, :hh, :, CT:], tmp[:, :hh, :, CT:],
                            tmp[:, hh : 2 * hh, :, CT:]
                        )
                        nc.vector.tensor_add(
                            tmp[:, :hh, :, :CT], tmp[:, :hh, :, :CT],
                            tmp[:, hh : 2 * hh, :, :CT]
                        )
                    for hh in (4, 2):
                        nc.vector.tensor_add(
                            tmp[:, :hh], tmp[:, :hh], tmp[:, hh : 2 * hh]
                        )
                    nc.vector.tensor_add(S[:], tmp[:, 0], tmp[:, 1])
                    yield
                    nc.gpsimd.tensor_add(S[:], S[:], bias_nc)
                    yield from squash(t, S[:], it,
                                      f32 if (it == 3 or CFG["skip_iter3"])
                                      else bf16)
                    V = squash.out
                    if it == 2:
                        # ---- distances -> logits ----
                        tmp = bigp.tile([128, CI, NO, CO], bf16, tag="tmp",
                                        name=f"tmpd{t}")
                        bigmul(tmp, U, lambda lo, hi: V[:, :, lo:hi]
                               .unsqueeze(1)
                               .broadcast_to([128, CI, NO, hi - lo]))
                        yield
                        D = statep.tile([128, CI, CO], bf16, tag="E",
                                        name=f"D{t}")
                        d_tree(tmp, D[:])
                        nc.vector.tensor_add(L[:], L[:], D[:])
                        yield
                emit_out(t, V)

            def drain(gens):
                alive = [g for g in gens if g is not None]
                while alive:
                    for g in list(alive):
                        try:
                            next(g)
                        except StopIteration:
                            alive.remove(g)

            G = CFG["group"]
            if G > 1:
                # group 0's conv up front; each group's routing drains
                # together with the NEXT group's conv generators (buffer
                # reuse, not emission order, staggers the conv work)
                starts = list(range(0, TILES, G))
                drain([conv_tile(t) for t in range(min(G, TILES))])
                for gi, s0 in enumerate(starts):
                    ts_ = range(s0, min(s0 + G, TILES))
                    gens = [routing_tile(t, *conv_tile.out[t]) for t in ts_]
                    if gi + 1 < len(starts):
                        n0 = starts[gi + 1]
                        gens += [conv_tile(t)
                                 for t in range(n0, min(n0 + G, TILES))]
                    drain(gens)
            else:
                for t in range(TILES):
                    drain([conv_tile(t)])
                    drain([routing_tile(t, *conv_tile.out[t])])



    nc.compile()
    _BUILt[key] = nc
    return nc


def _assemble(out_halves_all):
    o = out_halves_all.reshape(-1, 2, 4, CO, NPIX)
    return np.ascontiguousarray(
        o.transpose(0, 3, 1, 2, 4).reshape(-1, CO, NO, H, W)
    )


def kernel(x, conv_w, bias):
    import sys
    if "/opt/trn_rl_repo" not in sys.path:
        sys.path.insert(0, "/opt/trn_rl_repo")
    from concourse import bass_utils

    patches, w_m, bias_bc, ident = _host_prep(x, conv_w, bias)
    nc = _build_nc()
    in_maps = [
        {"patches": patches[b], "w": w_m, "bias": bias_bc, "ident": ident}
        for b in range(BS)
    ]
    res = bass_utils.run_bass_kernel_spmd(nc, in_maps, core_ids=list(range(BS)))
    outs = np.stack([r["out"] for r in res.results])
    return _assemble(outs).astype(np.float32)

